# revision 7
# baseline (speedup 1.0000x reference)
"""Per-pixel dynamic 5x5 conv (KernelConv2d) + leaky-relu, data-parallel on 8 TRN2 cores.

Sharding: core i <- (n = i//2, h-half = i%2); each core computes out[n, :, h0:h0+128, :].

v2 design (TensorE-accumulate): the baseline was DVE-bound (49 tensor_tensor ops
~60us). Here DVE computes ONLY the 25 per-tap elementwise products (x-window *
kernel-plane) in x-row partition space; all accumulation (the dy partition shift
+ the 25-tap sum) runs on the otherwise-idle TensorE as shifted-identity matmuls
accumulating in fp32 PSUM. ScalarE evicts PSUM->SBUF with fused leaky-relu.
x is loaded ONCE (plus a second 1-col-shifted copy for 4B alignment of odd-dx
reads), eliminating the baseline's 5x row duplication: HBM traffic drops from
~19MB to ~15.3MB per core, and the three compute engines each stay below the
DMA roofline (~43us at 358 GB/s).

Partition layout: partition q = padded x row h0+q (q=0..127). Product plane for
tap (dy,dx): P[q] = x[q, w+dx] * k[dy,dx][row h0+q-dy] (kernel plane pre-shifted
host-side, zero rows where q<dy). Shift_dy[q, p] = 1 iff p == q-dy routes row q
to output row p and matmul-accumulates over taps. Output rows p with p+dy > 127
(x rows 128..131) are covered by a 50-partition host-gathered tail product +
one extra matmul with a scatter matrix.
"""

import os
from contextlib import ExitStack

import numpy as np

import concourse.bass as bass
import concourse.mybir as mybir
from concourse.bass_utils import run_bass_kernel_spmd

N, C, H, W = 4, 8, 256, 256
K = 5
PAD = 2
NCORES = 8
HSH = H // 2            # 128 output rows per core
XW = 264                # stored x row width per alignment copy
CD = mybir.dt.float16
NEG = 0.2
NB = 4                  # product ring buffers
NTAP = K * K            # 25
NPAIR = 13              # kernel-plane DMA granularity: 2 taps per DMA (last is 1)
# tail (dy, p) combos: output rows needing x rows 128..131
COMBOS = [(dy, p) for dy in (1, 2, 3, 4) for p in range(HSH - dy, HSH)]
NTAIL = len(COMBOS) * K  # 50

_NC_CACHE = {}


def _build_nc():
    nc = bass.Bass("TRN2", target_bir_lowering=False, debug=False,
                   num_devices=NCORES)
    xs_d = nc.dram_tensor("xs", [HSH, 2, C, XW], CD, kind="ExternalInput").ap()
    wt_d = nc.dram_tensor("wt", [HSH, 6, HSH], CD, kind="ExternalInput").ap()
    xt_d = nc.dram_tensor("xt", [NTAIL, C, W], CD, kind="ExternalInput").ap()
    ktl_d = nc.dram_tensor("ktl", [NTAIL, C, W], CD, kind="ExternalInput").ap()
    kp_d = nc.dram_tensor("kp", [HSH, NTAP, C, W], CD, kind="ExternalInput").ap()
    out_d = nc.dram_tensor("out", [HSH, C, W], CD, kind="ExternalOutput").ap()

    with ExitStack() as ctx:
        xs = ctx.enter_context(nc.sbuf_tensor("xs_s", [HSH, 2, C, XW], CD))
        wt = ctx.enter_context(nc.sbuf_tensor("wt_s", [HSH, 6, HSH], CD))
        xt = ctx.enter_context(nc.sbuf_tensor("xt_s", [NTAIL, C, W], CD))
        ktl = ctx.enter_context(nc.sbuf_tensor("ktl_s", [NTAIL, C, W], CD))
        kt = ctx.enter_context(nc.sbuf_tensor("kt_s", [HSH, NTAP, C, W], CD))
        prod = [ctx.enter_context(nc.sbuf_tensor(f"pr{b}", [HSH, C, W], CD))
                for b in range(NB)]
        ptail = ctx.enter_context(nc.sbuf_tensor("ptail", [NTAIL, C, W], CD))
        tmp = ctx.enter_context(nc.sbuf_tensor("tmp", [HSH, C, W], CD))
        ot = ctx.enter_context(nc.sbuf_tensor("ot", [HSH, C, W], CD))
        pt = ctx.enter_context(nc.psum_tensor("pt", [HSH, C, W], mybir.dt.float32))

        s_x = ctx.enter_context(nc.semaphore("s_x"))
        s_w = ctx.enter_context(nc.semaphore("s_w"))
        s_xt = ctx.enter_context(nc.semaphore("s_xt"))
        s_kt = ctx.enter_context(nc.semaphore("s_kt"))
        s_k = [ctx.enter_context(nc.semaphore(f"s_k{j}")) for j in range(NPAIR)]
        s_v = ctx.enter_context(nc.semaphore("s_v"))    # DVE products done (count)
        s_mm = ctx.enter_context(nc.semaphore("s_mm"))  # PE tap-groups done (count)
        s_c = ctx.enter_context(nc.semaphore("s_c"))    # final per-chunk MMs done
        s_t = ctx.enter_context(nc.semaphore("s_t"))    # 0.2*x chunks done
        s_e = ctx.enter_context(nc.semaphore("s_e"))    # evict chunks done
        s_o = ctx.enter_context(nc.semaphore("s_o"))
        block = ctx.enter_context(nc.Block())

        @block.sync
        def _(sync):
            sync.dma_start(xt[:], xt_d).then_inc(s_xt, 16)
            sync.dma_start(ktl[:], ktl_d).then_inc(s_kt, 16)
            sync.dma_start(xs[:], xs_d).then_inc(s_x, 16)
            sync.dma_start(wt[:], wt_d).then_inc(s_w, 16)
            for j in range(NPAIR):
                t0, t1 = 2 * j, min(2 * j + 2, NTAP)
                sync.dma_start(kt[:, t0:t1], kp_d[:, t0:t1]).then_inc(s_k[j], 16)
            # output in halves, each after its two evict chunks complete
            sync.wait_ge(s_e, 2)
            sync.dma_start(out_d[:, 0:4], ot[:, 0:4]).then_inc(s_o, 16)
            sync.wait_ge(s_e, 4)
            sync.dma_start(out_d[:, 4:8], ot[:, 4:8]).then_inc(s_o, 16)
            sync.wait_ge(s_o, 32)

        @block.vector
        def _(vector):
            # queue position 0: tail product (own buffer, no reuse wait)
            vector.wait_ge(s_xt, 16)
            vector.wait_ge(s_kt, 16)
            vector.tensor_tensor(ptail[0:NTAIL], xt[0:NTAIL], ktl[0:NTAIL],
                                 op=mybir.AluOpType.mult).then_inc(s_v, 1)
            vector.wait_ge(s_x, 16)
            for i in range(NTAP):
                dy, dx = divmod(i, K)
                a = dx & 1
                off = dx - a
                if i % 2 == 0:
                    vector.wait_ge(s_k[i // 2], 16)
                if i >= NB:
                    # buffer prod[i%NB] free once PE finished queue item of
                    # tap i-NB (queue position (i-NB)+1; +1 for the tail)
                    vector.wait_ge(s_mm, i - NB + 2)
                vector.tensor_tensor(prod[i % NB][:],
                                     xs[:, a, :, off:off + W],
                                     kt[:, i],
                                     op=mybir.AluOpType.mult).then_inc(s_v, 1)
            # leaky-relu: max(0.2*x, x) with 0.2*x from ScalarE, x from PSUM
            for c in range(4):
                vector.wait_ge(s_t, c + 1)
                vector.tensor_tensor(ot[:, 2 * c:2 * c + 2],
                                     tmp[:, 2 * c:2 * c + 2],
                                     pt[:, 2 * c:2 * c + 2],
                                     op=mybir.AluOpType.max).then_inc(s_e, 1)

        @block.tensor
        def _(tensor):
            tensor.wait_ge(s_w, 16)
            # tail first: start=True initializes each PSUM chunk
            tensor.wait_ge(s_v, 1)
            for c in range(4):
                mm = tensor.matmul(pt[:, 2 * c:2 * c + 2],
                                   lhsT=wt[0:NTAIL, 5],
                                   rhs=ptail[0:NTAIL, 2 * c:2 * c + 2],
                                   start=True, stop=False)
                if c == 3:
                    mm.then_inc(s_mm, 1)
            for i in range(NTAP):
                dy = i // K
                tensor.wait_ge(s_v, i + 2)
                last = i == NTAP - 1
                for c in range(4):
                    mm = tensor.matmul(pt[:, 2 * c:2 * c + 2],
                                       lhsT=wt[:, dy],
                                       rhs=prod[i % NB][:, 2 * c:2 * c + 2],
                                       start=False, stop=last)
                    if last:
                        mm.then_inc(s_c, 1)
                    elif c == 3:
                        mm.then_inc(s_mm, 1)

        @block.scalar
        def _(scalar):
            for c in range(4):
                scalar.wait_ge(s_c, c + 1)
                scalar.activation(tmp[:, 2 * c:2 * c + 2], pt[:, 2 * c:2 * c + 2],
                                  mybir.ActivationFunctionType.Copy,
                                  bias=0.0, scale=NEG).then_inc(s_t, 1)
    return nc


def get_nc():
    if "nc" not in _NC_CACHE:
        _NC_CACHE["nc"] = _build_nc()
    return _NC_CACHE["nc"]


def _prep_shards(x: np.ndarray, kernel: np.ndarray):
    """Host-side: pad, cast to fp16, build per-core DMA layouts."""
    f16 = np.float16
    # pad rows 2 each side; cols 2 left, 7 right (reads use cols 0..264)
    xp = np.pad(x, ((0, 0), (0, 0), (PAD, PAD), (PAD, XW + 1 - W - PAD)),
                mode='edge').astype(f16)  # (N, C, 260, 265)
    kr = kernel.reshape(N, C, NTAP, H, W)

    in_maps = []
    for core in range(NCORES):
        n, hb = divmod(core, 2)
        h0 = hb * HSH
        blk = xp[n, :, h0:h0 + HSH + 4, :]          # (C, 132, 265)
        xsb = np.empty((HSH, 2, C, XW), f16)
        xsb[:, 0] = blk[:, :HSH, 0:XW].transpose(1, 0, 2)
        xsb[:, 1] = blk[:, :HSH, 1:XW + 1].transpose(1, 0, 2)

        kp = np.zeros((HSH, NTAP, C, W), f16)
        kb = kr[n, :, :, h0:h0 + HSH, :].astype(f16)  # (C, 25, 128, W)
        for t in range(NTAP):
            dy = t // K
            kp[dy:, t] = kb[:, t, :HSH - dy].transpose(1, 0, 2)

        xtb = np.zeros((NTAIL, C, W), f16)
        ktb = np.zeros((NTAIL, C, W), f16)
        for j, (dy, p) in enumerate(COMBOS):
            for dx in range(K):
                xtb[j * K + dx] = blk[:, p + dy, dx:dx + W]
                ktb[j * K + dx] = kb[:, dy * K + dx, p]

        wtb = np.zeros((HSH, 6, HSH), f16)
        for dy in range(K):
            q = np.arange(dy, HSH)
            wtb[q, dy, q - dy] = 1.0
        for j, (dy, p) in enumerate(COMBOS):
            for dx in range(K):
                wtb[j * K + dx, 5, p] = 1.0

        in_maps.append({"xs": xsb, "wt": wtb, "xt": xtb, "ktl": ktb, "kp": kp})
    return in_maps


def kernel(x: np.ndarray, kernel: np.ndarray) -> np.ndarray:
    nc = get_nc()
    in_maps = _prep_shards(np.asarray(x), np.asarray(kernel))
    trace = bool(int(os.environ.get("KC_TRACE", "0")))
    res = run_bass_kernel_spmd(nc, in_maps, core_ids=list(range(NCORES)),
                               trace=trace)
    _NC_CACHE["last_results"] = res
    out = np.empty((N, C, H, W), np.float32)
    for core in range(NCORES):
        n, hb = divmod(core, 2)
        h0 = hb * HSH
        o = res.results[core]["out"]  # (128, C, W) fp16
        out[n, :, h0:h0 + HSH, :] = o.transpose(1, 0, 2).astype(np.float32)
    return out


# revision 8
# speedup vs baseline: 1.0191x; 1.0191x over previous
"""Per-pixel dynamic 5x5 conv (KernelConv2d) + leaky-relu, data-parallel on 8 TRN2 cores.

Sharding: core i <- (n = i//2, h-half = i%2); each core computes out[n, :, h0:h0+128, :].

v2 design (TensorE-accumulate): the baseline was DVE-bound (49 tensor_tensor ops
~60us). Here DVE computes ONLY the 25 per-tap elementwise products (x-window *
kernel-plane) in x-row partition space; all accumulation (the dy partition shift
+ the 25-tap sum) runs on the otherwise-idle TensorE as shifted-identity matmuls
accumulating in fp32 PSUM. ScalarE evicts PSUM->SBUF with fused leaky-relu.
x is loaded ONCE (plus a second 1-col-shifted copy for 4B alignment of odd-dx
reads), eliminating the baseline's 5x row duplication: HBM traffic drops from
~19MB to ~15.3MB per core, and the three compute engines each stay below the
DMA roofline (~43us at 358 GB/s).

Partition layout: partition q = padded x row h0+q (q=0..127). Product plane for
tap (dy,dx): P[q] = x[q, w+dx] * k[dy,dx][row h0+q-dy] (kernel plane pre-shifted
host-side, zero rows where q<dy). Shift_dy[q, p] = 1 iff p == q-dy routes row q
to output row p and matmul-accumulates over taps. Output rows p with p+dy > 127
(x rows 128..131) are covered by a 50-partition host-gathered tail product +
one extra matmul with a scatter matrix.
"""

import os
from contextlib import ExitStack

import numpy as np

import concourse.bass as bass
import concourse.mybir as mybir
from concourse.bass_utils import run_bass_kernel_spmd

N, C, H, W = 4, 8, 256, 256
K = 5
PAD = 2
NCORES = 8
HSH = H // 2            # 128 output rows per core
XW = 264                # stored x row width per alignment copy
CD = mybir.dt.float16
NEG = 0.2
NB = 4                  # product ring buffers
NTAP = K * K            # 25
NPAIR = 13              # kernel-plane DMA granularity: 2 taps per DMA (last is 1)
# tail (dy, p) combos: output rows needing x rows 128..131
COMBOS = [(dy, p) for dy in (1, 2, 3, 4) for p in range(HSH - dy, HSH)]
NTAIL = len(COMBOS) * K  # 50

_NC_CACHE = {}


def _build_nc():
    nc = bass.Bass("TRN2", target_bir_lowering=False, debug=False,
                   num_devices=NCORES)
    xs_d = nc.dram_tensor("xs", [HSH, 2, C, XW], CD, kind="ExternalInput").ap()
    wt_d = nc.dram_tensor("wt", [HSH, 6, HSH], CD, kind="ExternalInput").ap()
    xt_d = nc.dram_tensor("xt", [NTAIL, C, W], CD, kind="ExternalInput").ap()
    ktl_d = nc.dram_tensor("ktl", [NTAIL, C, W], CD, kind="ExternalInput").ap()
    kp_d = nc.dram_tensor("kp", [HSH, NTAP, C, W], CD, kind="ExternalInput").ap()
    out_d = nc.dram_tensor("out", [HSH, C, W], CD, kind="ExternalOutput").ap()

    with ExitStack() as ctx:
        xs = ctx.enter_context(nc.sbuf_tensor("xs_s", [HSH, 2, C, XW], CD))
        wt = ctx.enter_context(nc.sbuf_tensor("wt_s", [HSH, 6, HSH], CD))
        xt = ctx.enter_context(nc.sbuf_tensor("xt_s", [NTAIL, C, W], CD))
        ktl = ctx.enter_context(nc.sbuf_tensor("ktl_s", [NTAIL, C, W], CD))
        kt = ctx.enter_context(nc.sbuf_tensor("kt_s", [HSH, NTAP, C, W], CD))
        prod = [ctx.enter_context(nc.sbuf_tensor(f"pr{b}", [HSH, C, W], CD))
                for b in range(NB)]
        ptail = ctx.enter_context(nc.sbuf_tensor("ptail", [NTAIL, C, W], CD))
        tmp = ctx.enter_context(nc.sbuf_tensor("tmp", [HSH, C, W], CD))
        ot = ctx.enter_context(nc.sbuf_tensor("ot", [HSH, C, W], CD))
        pt = ctx.enter_context(nc.psum_tensor("pt", [HSH, C, W], mybir.dt.float32))

        s_x = ctx.enter_context(nc.semaphore("s_x"))
        s_w = ctx.enter_context(nc.semaphore("s_w"))
        s_xt = ctx.enter_context(nc.semaphore("s_xt"))
        s_kt = ctx.enter_context(nc.semaphore("s_kt"))
        s_k = [ctx.enter_context(nc.semaphore(f"s_k{j}")) for j in range(NPAIR)]
        s_v = ctx.enter_context(nc.semaphore("s_v"))    # DVE products done (count)
        s_mm = ctx.enter_context(nc.semaphore("s_mm"))  # PE tap-groups done (count)
        s_c = ctx.enter_context(nc.semaphore("s_c"))    # final per-chunk MMs done
        s_t = ctx.enter_context(nc.semaphore("s_t"))    # 0.2*x chunks done
        s_e = ctx.enter_context(nc.semaphore("s_e"))    # evict chunks done
        s_o = ctx.enter_context(nc.semaphore("s_o"))
        block = ctx.enter_context(nc.Block())

        @block.sync
        def _(sync):
            sync.dma_start(xt[:], xt_d).then_inc(s_xt, 16)
            sync.dma_start(ktl[:], ktl_d).then_inc(s_kt, 16)
            sync.dma_start(xs[:], xs_d).then_inc(s_x, 16)
            sync.dma_start(wt[:], wt_d).then_inc(s_w, 16)
            for j in range(NPAIR):
                t0, t1 = 2 * j, min(2 * j + 2, NTAP)
                sync.dma_start(kt[:, t0:t1], kp_d[:, t0:t1]).then_inc(s_k[j], 16)
            # output in halves, each after its two evict chunks complete
            sync.wait_ge(s_e, 2)
            sync.dma_start(out_d[:, 0:4], ot[:, 0:4]).then_inc(s_o, 16)
            sync.wait_ge(s_e, 4)
            sync.dma_start(out_d[:, 4:8], ot[:, 4:8]).then_inc(s_o, 16)
            sync.wait_ge(s_o, 32)

        @block.vector
        def _(vector):
            # queue position 0: tail product (own buffer, no reuse wait)
            vector.wait_ge(s_xt, 16)
            vector.wait_ge(s_kt, 16)
            vector.tensor_tensor(ptail[0:NTAIL], xt[0:NTAIL], ktl[0:NTAIL],
                                 op=mybir.AluOpType.mult).then_inc(s_v, 1)
            vector.wait_ge(s_x, 16)
            for i in range(NTAP):
                dy, dx = divmod(i, K)
                a = dx & 1
                off = dx - a
                if i % 2 == 0:
                    vector.wait_ge(s_k[i // 2], 16)
                if i >= NB:
                    # buffer prod[i%NB] free once PE finished queue item of
                    # tap i-NB (queue position (i-NB)+1; +1 for the tail)
                    vector.wait_ge(s_mm, i - NB + 2)
                vector.tensor_tensor(prod[i % NB][:],
                                     xs[:, a, :, off:off + W],
                                     kt[:, i],
                                     op=mybir.AluOpType.mult).then_inc(s_v, 1)
            # leaky-relu: max(0.2*x, x) with 0.2*x from ScalarE, x from PSUM
            for c in range(4):
                vector.wait_ge(s_t, c + 1)
                vector.tensor_tensor(ot[:, 2 * c:2 * c + 2],
                                     tmp[:, 2 * c:2 * c + 2],
                                     pt[:, 2 * c:2 * c + 2],
                                     op=mybir.AluOpType.max).then_inc(s_e, 1)

        @block.tensor
        def _(tensor):
            tensor.wait_ge(s_w, 16)
            # tail first: start=True initializes each PSUM chunk
            tensor.wait_ge(s_v, 1)
            for c in range(4):
                mm = tensor.matmul(pt[:, 2 * c:2 * c + 2],
                                   lhsT=wt[0:NTAIL, 5],
                                   rhs=ptail[0:NTAIL, 2 * c:2 * c + 2],
                                   start=True, stop=False)
                if c > 0:
                    mm.ins.ldweights = False  # weights unchanged within group
                if c == 3:
                    mm.then_inc(s_mm, 1)
            for i in range(NTAP):
                dy = i // K
                tensor.wait_ge(s_v, i + 2)
                last = i == NTAP - 1
                for c in range(4):
                    mm = tensor.matmul(pt[:, 2 * c:2 * c + 2],
                                       lhsT=wt[:, dy],
                                       rhs=prod[i % NB][:, 2 * c:2 * c + 2],
                                       start=False, stop=last)
                    if i % K != 0 or c > 0:
                        mm.ins.ldweights = False  # same dy -> same weights
                    if last:
                        mm.then_inc(s_c, 1)
                    elif c == 3:
                        mm.then_inc(s_mm, 1)

        @block.scalar
        def _(scalar):
            for c in range(4):
                scalar.wait_ge(s_c, c + 1)
                scalar.activation(tmp[:, 2 * c:2 * c + 2], pt[:, 2 * c:2 * c + 2],
                                  mybir.ActivationFunctionType.Copy,
                                  bias=0.0, scale=NEG).then_inc(s_t, 1)
    return nc


def get_nc():
    if "nc" not in _NC_CACHE:
        _NC_CACHE["nc"] = _build_nc()
    return _NC_CACHE["nc"]


def _prep_shards(x: np.ndarray, kernel: np.ndarray):
    """Host-side: pad, cast to fp16, build per-core DMA layouts."""
    f16 = np.float16
    # pad rows 2 each side; cols 2 left, 7 right (reads use cols 0..264)
    xp = np.pad(x, ((0, 0), (0, 0), (PAD, PAD), (PAD, XW + 1 - W - PAD)),
                mode='edge').astype(f16)  # (N, C, 260, 265)
    kr = kernel.reshape(N, C, NTAP, H, W)

    in_maps = []
    for core in range(NCORES):
        n, hb = divmod(core, 2)
        h0 = hb * HSH
        blk = xp[n, :, h0:h0 + HSH + 4, :]          # (C, 132, 265)
        xsb = np.empty((HSH, 2, C, XW), f16)
        xsb[:, 0] = blk[:, :HSH, 0:XW].transpose(1, 0, 2)
        xsb[:, 1] = blk[:, :HSH, 1:XW + 1].transpose(1, 0, 2)

        kp = np.zeros((HSH, NTAP, C, W), f16)
        kb = kr[n, :, :, h0:h0 + HSH, :].astype(f16)  # (C, 25, 128, W)
        for t in range(NTAP):
            dy = t // K
            kp[dy:, t] = kb[:, t, :HSH - dy].transpose(1, 0, 2)

        xtb = np.zeros((NTAIL, C, W), f16)
        ktb = np.zeros((NTAIL, C, W), f16)
        for j, (dy, p) in enumerate(COMBOS):
            for dx in range(K):
                xtb[j * K + dx] = blk[:, p + dy, dx:dx + W]
                ktb[j * K + dx] = kb[:, dy * K + dx, p]

        wtb = np.zeros((HSH, 6, HSH), f16)
        for dy in range(K):
            q = np.arange(dy, HSH)
            wtb[q, dy, q - dy] = 1.0
        for j, (dy, p) in enumerate(COMBOS):
            for dx in range(K):
                wtb[j * K + dx, 5, p] = 1.0

        in_maps.append({"xs": xsb, "wt": wtb, "xt": xtb, "ktl": ktb, "kp": kp})
    return in_maps


def kernel(x: np.ndarray, kernel: np.ndarray) -> np.ndarray:
    nc = get_nc()
    in_maps = _prep_shards(np.asarray(x), np.asarray(kernel))
    trace = bool(int(os.environ.get("KC_TRACE", "0")))
    res = run_bass_kernel_spmd(nc, in_maps, core_ids=list(range(NCORES)),
                               trace=trace)
    _NC_CACHE["last_results"] = res
    out = np.empty((N, C, H, W), np.float32)
    for core in range(NCORES):
        n, hb = divmod(core, 2)
        h0 = hb * HSH
        o = res.results[core]["out"]  # (128, C, W) fp16
        out[n, :, h0:h0 + HSH, :] = o.transpose(1, 0, 2).astype(np.float32)
    return out


# revision 21
# speedup vs baseline: 1.0422x; 1.0227x over previous
"""Per-pixel dynamic 5x5 conv (KernelConv2d) + leaky-relu, data-parallel on 8 TRN2 cores.

Sharding: core i <- (n = i//2, h-half = i%2); each core computes out[n, :, h0:h0+128, :].

v3 design (TensorE-accumulate, column-halved):
- DVE computes ONLY the 25 per-tap elementwise products (x-window * kernel
  plane) in x-row partition space; the dy partition-shift and the 25-tap sum
  run on TensorE as shifted-identity matmuls accumulating in fp32 PSUM
  (ldweights skipped when consecutive matmuls share weights). ScalarE scales
  0.2*x out of PSUM, DVE finishes leaky-relu with max(0.2x, x).
- x is loaded ONCE (plus a 1-col-shifted copy for 4B alignment of odd dx);
  no 5x row duplication: ~15.3MB HBM per core vs baseline ~19MB.
- The entire pipeline is split into two column halves (w<128 / w>=128): the
  kernel stream delivers all taps of half 0 first, so half 0 accumulates,
  evicts, and DMAs out while half 1 is still streaming -- the post-stream
  tail is only half 1's last tap + a short evict chain.
- Output rows p with p+dy > 127 (x rows 128..131) are covered by a
  50-partition host-gathered tail product + one scatter matmul per half.

Partition layout: partition q = padded x row h0+q (q=0..127). Product plane for
tap (dy,dx): P[q] = x[q, w+dx] * k[dy,dx][row h0+q-dy] (kernel plane pre-shifted
host-side, zero rows where q<dy). Shift_dy[q, p] = 1 iff p == q-dy routes row q
to output row p and matmul-accumulates over taps.
"""

import os
from contextlib import ExitStack

import numpy as np

import concourse.bass as bass
import concourse.mybir as mybir
from concourse.bass_utils import run_bass_kernel_spmd

N, C, H, W = 4, 8, 256, 256
K = 5
PAD = 2
NCORES = 8
HSH = H // 2            # 128 output rows per core
XW = 264                # stored x row width per alignment copy
HW2 = W // 2            # 128: column half width
CD = mybir.dt.float16
NEG = 0.2
NB = 8                  # product ring buffers
NWARM = 6               # PE warm-up dummy matmuls (HAM clock-gate)
NTAP = K * K            # 25
NPAIR = 13              # kernel DMA granularity: 2 taps per DMA per half
COMBOS = [(dy, p) for dy in (1, 2, 3, 4) for p in range(HSH - dy, HSH)]
NTAIL = len(COMBOS) * K  # 50

_NC_CACHE = {}


def _build_nc():
    nc = bass.Bass("TRN2", target_bir_lowering=False, debug=False,
                   num_devices=NCORES)
    # aux: x windows (2 alignments) + the 6 shift/scatter matrices
    aux_d = nc.dram_tensor("aux", [HSH, 2 * C * XW + 6 * HSH], CD,
                           kind="ExternalInput").ap()
    tl_d = nc.dram_tensor("tl", [NTAIL, 2, C, W], CD, kind="ExternalInput").ap()
    kp_d = nc.dram_tensor("kp", [HSH, 2, NTAP, C, HW2], CD,
                          kind="ExternalInput").ap()
    out_d = nc.dram_tensor("out", [HSH, 2, C, HW2], CD, kind="ExternalOutput").ap()

    with ExitStack() as ctx:
        aux = ctx.enter_context(
            nc.sbuf_tensor("aux_s", [HSH, 2 * C * XW + 6 * HSH], CD))
        tl = ctx.enter_context(nc.sbuf_tensor("tl_s", [NTAIL, 2, C, W], CD))
        kt = ctx.enter_context(nc.sbuf_tensor("kt_s", [HSH, 2, NTAP, C, HW2], CD))
        prod = [ctx.enter_context(nc.sbuf_tensor(f"pr{b}", [HSH, C, HW2], CD))
                for b in range(NB)]
        ptail = ctx.enter_context(nc.sbuf_tensor("ptail", [NTAIL, 2, C, HW2], CD))
        tmp = ctx.enter_context(nc.sbuf_tensor("tmp", [HSH, 2, C, HW2], CD))
        ot = ctx.enter_context(nc.sbuf_tensor("ot", [HSH, 2, C, HW2], CD))
        pt = ctx.enter_context(
            nc.psum_tensor("pt", [HSH, 2, C, HW2], mybir.dt.float32))
        scr = ctx.enter_context(
            nc.psum_tensor("scr", [HSH, 512], mybir.dt.float32))

        xs = aux[:, 0:2 * C * XW].rearrange("p (a c x) -> p a c x", a=2, c=C)
        wt = aux[:, 2 * C * XW:].rearrange("p (g m) -> p g m", g=6)

        s_a = ctx.enter_context(nc.semaphore("s_a"))
        s_tl = ctx.enter_context(nc.semaphore("s_tl"))
        s_k = [ctx.enter_context(nc.semaphore(f"s_k{j}"))
               for j in range(2 * NPAIR)]
        s_v = ctx.enter_context(nc.semaphore("s_v"))    # DVE products done
        s_mm = ctx.enter_context(nc.semaphore("s_mm"))  # PE tap-groups done
        s_c = ctx.enter_context(nc.semaphore("s_c"))    # per-half final MMs
        s_t = ctx.enter_context(nc.semaphore("s_t"))    # 0.2*x chunks done
        s_e = ctx.enter_context(nc.semaphore("s_e"))    # lrelu chunks done
        s_o = ctx.enter_context(nc.semaphore("s_o"))
        block = ctx.enter_context(nc.Block())

        # vector/PE queue: [tail_h0, tail_h1] + 50 half-taps (h0 taps then h1).
        # PE consumes products in pairs (t = seq//2); s_mm hits t+2 when pair t
        # is consumed (pair 24 increments s_c instead, never waited via s_mm).
        def smm_after(j):  # s_mm value guaranteeing half-tap seq j is consumed
            t = j // 2
            assert t < 24
            return t + 2

        @block.sync
        def _(sync):
            sync.dma_start(tl[:], tl_d).then_inc(s_tl, 16)
            sync.dma_start(aux[:], aux_d).then_inc(s_a, 16)
            for h in range(2):
                for j in range(NPAIR):
                    t0, t1 = 2 * j, min(2 * j + 2, NTAP)
                    sync.dma_start(kt[:, h, t0:t1],
                                   kp_d[:, h, t0:t1]).then_inc(
                                       s_k[h * NPAIR + j], 16)
            sync.wait_ge(s_o, 32)

        @block.vector
        def _(vector):
            vector.wait_ge(s_tl, 16)
            for h in range(2):
                vector.tensor_tensor(ptail[0:NTAIL, h],
                                     tl[0:NTAIL, 0, :, h * HW2:(h + 1) * HW2],
                                     tl[0:NTAIL, 1, :, h * HW2:(h + 1) * HW2],
                                     op=mybir.AluOpType.mult).then_inc(s_v, 1)
            vector.wait_ge(s_a, 16)

            def lrelu_max(vector, h, q):
                vector.wait_ge(s_t, 2 * h + q + 1)
                vector.tensor_tensor(ot[:, h, 4 * q:4 * q + 4],
                                     tmp[:, h, 4 * q:4 * q + 4],
                                     pt[:, h, 4 * q:4 * q + 4],
                                     op=mybir.AluOpType.max).then_inc(s_e, 1)

            for h in range(2):
                for i in range(NTAP):
                    seq = h * NTAP + i
                    dy, dx = divmod(i, K)
                    a = dx & 1
                    off = dx - a + h * HW2
                    if i % 2 == 0:
                        vector.wait_ge(s_k[h * NPAIR + i // 2], 16)
                    if seq >= NB and seq % 4 == 0:
                        # batched ring-reuse wait covering seq..seq+3
                        vector.wait_ge(s_mm, smm_after(seq - 5))
                    vector.tensor_tensor(prod[seq % NB][:],
                                         xs[:, a, :, off:off + HW2],
                                         kt[:, h, i],
                                         op=mybir.AluOpType.mult).then_inc(s_v, 1)
                    # half-0 lrelu slots mid-way through half-1's stream
                    if h == 1 and i == 3:
                        lrelu_max(vector, 0, 0)
                    if h == 1 and i == 5:
                        lrelu_max(vector, 0, 1)
            lrelu_max(vector, 1, 0)
            lrelu_max(vector, 1, 1)

        @block.tensor
        def _(tensor):
            tensor.wait_ge(s_a, 16)
            # dummy matmuls: engage the PE HAM clock-gate (~3.4us of activity
            # flips the PE from 1.2 to 2.4 GHz) before the real stream arrives
            for r in range(NWARM):
                mm = tensor.matmul(scr[:], lhsT=wt[:, 0],
                                   rhs=xs[:, 0, 0:2, 0:W],
                                   start=True, stop=True)
                if r > 0:
                    mm.ins.ldweights = False
            prev_w = [0]  # warmup loaded wt[:, 0]

            def mmul(rhs_ap, h, q, wid, start, stop):
                mm = tensor.matmul(pt[:, h, 4 * q:4 * q + 4],
                                   lhsT=(wt[0:NTAIL, 5] if wid == 5
                                         else wt[:, wid]),
                                   rhs=rhs_ap,
                                   start=start, stop=stop)
                if wid == prev_w[0]:
                    mm.ins.ldweights = False
                prev_w[0] = wid
                return mm

            tensor.wait_ge(s_v, 2)
            for h in range(2):
                for q in range(2):
                    mm = mmul(ptail[0:NTAIL, h, 4 * q:4 * q + 4], h, q, 5,
                              True, False)
            mm.then_inc(s_mm, 1)
            for t in range(NTAP):  # pair t covers seqs 2t, 2t+1
                tensor.wait_ge(s_v, 2 * t + 4)
                pair_last = None
                for seq in (2 * t, 2 * t + 1):
                    h, i = divmod(seq, NTAP)
                    dy = i // K
                    last = i == NTAP - 1
                    for q in range(2):
                        mm = mmul(prod[seq % NB][:, 4 * q:4 * q + 4],
                                  h, q, dy, False, last)
                        if last and q == 1:
                            mm.then_inc(s_c, 1)
                        elif not last:
                            pair_last = mm
                if pair_last is not None and t < NTAP - 1:
                    pair_last.then_inc(s_mm, 1)

        @block.scalar
        def _(scalar):
            # acts + output DMAs ride the ACT HWDGE ring, which is empty --
            # issuing outputs from sync would FIFO behind the whole input stream
            scalar.wait_ge(s_a, 16)
            # preload the activation table outside the critical path
            scalar.activation(tmp[:, 0, 0], xs[:, 0, 0, 0:HW2],
                              mybir.ActivationFunctionType.Copy,
                              bias=0.0, scale=1.0)
            for h in range(2):
                for q in range(2):
                    scalar.wait_ge(s_c, h + 1)
                    scalar.activation(tmp[:, h, 4 * q:4 * q + 4],
                                      pt[:, h, 4 * q:4 * q + 4],
                                      mybir.ActivationFunctionType.Copy,
                                      bias=0.0, scale=NEG).then_inc(s_t, 1)
                scalar.wait_ge(s_e, 2 * h + 2)
                scalar.dma_start(out_d[:, h], ot[:, h]).then_inc(s_o, 16)
    return nc


def get_nc():
    if "nc" not in _NC_CACHE:
        _NC_CACHE["nc"] = _build_nc()
    return _NC_CACHE["nc"]


def _prep_shards(x: np.ndarray, kernel: np.ndarray):
    """Host-side: pad, cast to fp16, build per-core DMA layouts."""
    f16 = np.float16
    xp = np.pad(x, ((0, 0), (0, 0), (PAD, PAD), (PAD, XW + 1 - W - PAD)),
                mode='edge').astype(f16)  # (N, C, 260, 265)
    kr = kernel.reshape(N, C, NTAP, H, W)

    in_maps = []
    for core in range(NCORES):
        n, hb = divmod(core, 2)
        h0 = hb * HSH
        blk = xp[n, :, h0:h0 + HSH + 4, :]          # (C, 132, 265)
        aux = np.zeros((HSH, 2 * C * XW + 6 * HSH), f16)
        xsb = aux[:, :2 * C * XW].reshape(HSH, 2, C, XW)
        xsb[:, 0] = blk[:, :HSH, 0:XW].transpose(1, 0, 2)
        xsb[:, 1] = blk[:, :HSH, 1:XW + 1].transpose(1, 0, 2)
        wtb = aux[:, 2 * C * XW:].reshape(HSH, 6, HSH)
        for dy in range(K):
            q = np.arange(dy, HSH)
            wtb[q, dy, q - dy] = 1.0
        for j, (dy, p) in enumerate(COMBOS):
            for dx in range(K):
                wtb[j * K + dx, 5, p] = 1.0

        kb = kr[n, :, :, h0:h0 + HSH, :].astype(f16)  # (C, 25, 128, W)
        kp = np.zeros((HSH, NTAP, C, W), f16)
        for t in range(NTAP):
            dy = t // K
            kp[dy:, t] = kb[:, t, :HSH - dy].transpose(1, 0, 2)
        # column-half-major: (q, half, tap, c, 128)
        kph = np.ascontiguousarray(
            kp.reshape(HSH, NTAP, C, 2, HW2).transpose(0, 3, 1, 2, 4))

        tlb = np.zeros((NTAIL, 2, C, W), f16)
        for j, (dy, p) in enumerate(COMBOS):
            for dx in range(K):
                tlb[j * K + dx, 0] = blk[:, p + dy, dx:dx + W]
                tlb[j * K + dx, 1] = kb[:, dy * K + dx, p]

        in_maps.append({"aux": aux, "tl": tlb, "kp": kph})
    return in_maps


def kernel(x: np.ndarray, kernel: np.ndarray) -> np.ndarray:
    nc = get_nc()
    in_maps = _prep_shards(np.asarray(x), np.asarray(kernel))
    trace = bool(int(os.environ.get("KC_TRACE", "0")))
    res = run_bass_kernel_spmd(nc, in_maps, core_ids=list(range(NCORES)),
                               trace=trace)
    _NC_CACHE["last_results"] = res
    out = np.empty((N, C, H, W), np.float32)
    for core in range(NCORES):
        n, hb = divmod(core, 2)
        h0 = hb * HSH
        o = res.results[core]["out"]  # (128, 2, C, 128) fp16
        o = o.transpose(2, 0, 1, 3).reshape(C, HSH, W)
        out[n, :, h0:h0 + HSH, :] = o.astype(np.float32)
    return out
